# revision 1
# baseline (speedup 1.0000x reference)
"""Multi-head attention (B=4, S=2048, D=1024, H=16) on 8 NeuronCores.

Sharding: core c -> (batch b = c//2, head-group g = c%2 of 8 heads).
Per-core: column-parallel fused qkv projection for its 8 heads,
flash-style attention (scores kept transposed: k on partitions so
softmax denominators come from a fused ones-column in the PV matmul),
row-parallel out-projection. The two partial outputs per batch are
summed on the host along with b_out.

Matmuls run as float32r (full-rate fp32 mode, ~1e-4 component error);
exp on ACT from PSUM; the 0/1 mask is applied as a uint8 multiply
after exp (exp(-1000) == 0 in fp32, so masking probs is exact).
"""
import sys

if "/opt/trn_rl_repo" not in sys.path:
    sys.path.insert(0, "/opt/trn_rl_repo")

import numpy as np

B, S, D, H = 4, 2048, 1024, 16
DH = D // H          # 64
HPC = H // 2         # 8 heads per core
CD = HPC * DH        # 512 local head-dims per core
NCORES = 8

_CACHE = {}


def _split_multiwait(nc):
    """walrus in this container accepts ONE sync wait per instruction;
    hoist extras onto injected same-engine EventSemaphore carriers."""
    import concourse.mybir as mybir

    for fn in nc.m.functions:
        for bb in fn.blocks:
            if not any(
                i.sync_info is not None and i.sync_info.on_wait
                and len(i.sync_info.on_wait) > 1
                for i in bb.instructions
            ):
                continue
            newlist = []
            for inst in bb.instructions:
                si = inst.sync_info
                if si is not None and si.on_wait and len(si.on_wait) > 1:
                    waits = list(si.on_wait)
                    for w in waits[:-1]:
                        ev = mybir.InstEventSemaphore(
                            name=nc.get_next_instruction_name(), ins=[], outs=[])
                        ev.engine = inst.engine
                        ev.sync_info = mybir.SyncInfo(on_wait=[w], on_update=[])
                        newlist.append(ev)
                    inst.sync_info = mybir.SyncInfo(
                        on_wait=[waits[-1]], on_update=list(si.on_update))
                newlist.append(inst)
            try:
                bb.instructions = newlist
            except Exception:
                bb.instructions.clear()
                bb.instructions.extend(newlist)


def _rep_rows(src_row, nrep, width):
    """DMA source AP repeating a [1, width] sbuf row nrep times
    (free-dim zero stride)."""
    import concourse.bass as bass

    return bass.AP(src_row.tensor, src_row.offset,
                   [list(src_row.ap[0]), [0, nrep], [1, width]])


def build_nc(s=S, attn_bf16=False):
    import concourse.bass as bass
    import concourse.mybir as mybir
    from concourse.tile import TileContext

    F32 = mybir.dt.float32
    F32R = mybir.dt.float32r
    BF16 = mybir.dt.bfloat16
    AT = BF16 if attn_bf16 else F32R
    U8 = mybir.dt.uint8
    EXP = mybir.ActivationFunctionType.Exp
    MULT = mybir.AluOpType.mult

    n_sc = s // 128            # s-chunks of 128
    n_st = s // 512            # s-tiles of 512
    n_kc = s // 128            # k chunks (128 each)
    fd_q = min(1024, s)        # q-tile width for attention inner loop
    n_qh = s // fd_q           # q tiles
    n_qn = fd_q // 512         # 512-wide matmuls per q tile
    VW = CD + HPC              # vones row-chunk width (8 heads x 65)

    nc = bass.Bass("TRN2", num_devices=NCORES)

    xT = nc.declare_dram_parameter("xT", [D, s], F32R, isOutput=False)
    wqk = nc.declare_dram_parameter("wqk", [D, 2 * CD], F32R, isOutput=False)
    wv = nc.declare_dram_parameter("wv", [D, CD], F32R, isOutput=False)
    bqk = nc.declare_dram_parameter("bqk", [1, 2 * CD], F32R, isOutput=False)
    bv = nc.declare_dram_parameter("bv", [1, CD], F32R, isOutput=False)
    m01 = nc.declare_dram_parameter("m01", [s, s], U8, isOutput=False)
    wout = nc.declare_dram_parameter("wout", [CD, D], F32R, isOutput=False)
    ones = nc.declare_dram_parameter("ones", [1, 512], F32R, isOutput=False)
    ones_at = nc.declare_dram_parameter("ones_at", [1, 512], AT, isOutput=False)
    sel_lo = nc.declare_dram_parameter("sel_lo", [1, 128], F32R, isOutput=False)
    sel_hi = nc.declare_dram_parameter("sel_hi", [1, 128], F32R, isOutput=False)
    y = nc.declare_dram_parameter("y", [s, D], F32, isOutput=True)

    with TileContext(nc) as tc:
        with tc.tile_pool(name="persist", bufs=1) as pp:
            qkT = pp.tile([128, 8 * s], AT, tag="qkT")       # [1024 c, s]
            vones = pp.tile([128, n_sc * VW], AT, tag="vones")
            sel_lo_t = pp.tile([1, 128], F32R, tag="sel_lo")
            sel_hi_t = pp.tile([1, 128], F32R, tag="sel_hi")
            nc.sync.dma_start(out=sel_lo_t[:], in_=sel_lo[:])
            nc.sync.dma_start(out=sel_hi_t[:], in_=sel_hi[:])

            # ---------------- phase A: qkv projection ----------------
            with tc.tile_pool(name="poolA", bufs=1) as pa, \
                 tc.tile_pool(name="poolAw", bufs=8) as paw, \
                 tc.tile_pool(name="psA", bufs=8, space="PSUM") as psA:
                xt = pa.tile([128, 8 * s], F32R, tag="xt")
                wvt = pa.tile([128, 8 * CD], F32R, tag="wvt")
                ones_row = pa.tile([1, 512], F32R, tag="ones")
                bqk_t = pa.tile([1, 2 * CD], F32R, tag="bqk")
                bv_t = pa.tile([1, CD], F32R, tag="bv")

                nc.sync.dma_start(out=ones_row[:], in_=ones[:])
                nc.sync.dma_start(out=bqk_t[:], in_=bqk[:])
                nc.sync.dma_start(out=bv_t[:], in_=bv[:])
                for dc in range(8):
                    nc.sync.dma_start(out=xt[:, dc * s:(dc + 1) * s],
                                      in_=xT[dc * 128:(dc + 1) * 128, :])
                    nc.sync.dma_start(out=wvt[:, dc * CD:(dc + 1) * CD],
                                      in_=wv[dc * 128:(dc + 1) * 128, :])
                # ones columns of vones (the rest is overwritten below)
                vones_cols = vones[:].rearrange(
                    "p (ch e) -> p ch e", e=DH + 1)[:, :, DH:DH + 1]
                ones_rep = bass.AP(ones_at, 0,
                                   [[0, 128], [1, n_sc * HPC], [1, 1]])
                nc.sync.dma_start(out=vones_cols, in_=ones_rep)

                # q/k: qkT[c, :] = (W.T x.T), c-tiles of 128
                for ct in range(8):
                    pst = [psA.tile([128, 512], F32, tag="pa",
                                    name=f"psqk_{ct}_{st}")
                           for st in range(n_st)]
                    for dc in range(8):
                        wt = paw.tile([128, 128], F32R, tag="wqk")
                        nc.scalar.dma_start(
                            out=wt[:],
                            in_=wqk[dc * 128:(dc + 1) * 128,
                                    ct * 128:(ct + 1) * 128])
                        for st in range(n_st):
                            nc.tensor.matmul(
                                pst[st][:],
                                lhsT=wt[:],
                                rhs=xt[:, dc * s + st * 512:
                                       dc * s + (st + 1) * 512],
                                start=(dc == 0), stop=False)
                    for st in range(n_st):
                        nc.tensor.matmul(
                            pst[st][:],
                            lhsT=bqk_t[0:1, ct * 128:(ct + 1) * 128],
                            rhs=ones_row[0:1, :],
                            start=False, stop=True)
                        nc.scalar.copy(
                            out=qkT[:, ct * s + st * 512:ct * s + (st + 1) * 512],
                            in_=pst[st][:])

                # v: natural [s, c] layout, s-chunks of 128, fused ones col
                for scg in range(n_sc // 4):
                    psv = [psA.tile([128, 512], F32, tag="pa",
                                    name=f"psv_{scg}_{i}")
                           for i in range(4)]
                    for dc in range(8):
                        for sci in range(4):
                            sc = scg * 4 + sci
                            nc.tensor.matmul(
                                psv[sci][:],
                                lhsT=xt[:, dc * s + sc * 128:
                                        dc * s + (sc + 1) * 128],
                                rhs=wvt[:, dc * CD:(dc + 1) * CD],
                                start=(dc == 0), stop=False)
                    for sci in range(4):
                        sc = scg * 4 + sci
                        nc.tensor.matmul(
                            psv[sci][:],
                            lhsT=ones_row[0:1, 0:128],
                            rhs=bv_t[0:1, :],
                            start=False, stop=True)
                        dst = vones[:, sc * VW:(sc + 1) * VW].rearrange(
                            "p (h e) -> p h e", e=DH + 1)[:, :, 0:DH]
                        src = psv[sci][:].rearrange("p (h e) -> p h e", e=DH)
                        nc.vector.tensor_copy(dst, src)

            # ---------------- phase B: attention ----------------
            with tc.tile_pool(name="poolB", bufs=1) as pb:
                ctxT = pb.tile([128, 4 * s], F32R, tag="ctxT")   # [512 c, s]
                with (
                    tc.tile_pool(name="poolBm", bufs=1) as pbm,
                    tc.tile_pool(name="poolE", bufs=4) as pe,
                    tc.tile_pool(name="poolBc", bufs=2) as pbc,
                    tc.tile_pool(name="psB_st", bufs=2, space="PSUM") as ps_st,
                    tc.tile_pool(name="psB_ctx", bufs=2, space="PSUM") as ps_ctx,
                ):
                    m01t = pbm.tile([128, n_kc * s], U8, tag="m01")
                    for kc in range(n_kc):
                        nc.sync.dma_start(out=m01t[:, kc * s:(kc + 1) * s],
                                          in_=m01[kc * 128:(kc + 1) * 128, :])
                    tt_idx = 0
                    for hp in range(4):
                        h0, h1 = 2 * hp, 2 * hp + 1
                        kt_off = (4 + hp) * s   # K pair c-tile offset in qkT
                        qt_off = hp * s         # Q pair c-tile offset
                        rs_p = pbc.tile([2 * n_qh, fd_q], F32R, tag="rsp",
                                        name=f"rs_{hp}")
                        rcp_p = pbc.tile([2 * n_qh, fd_q], F32R, tag="rcpp",
                                         name=f"rcp_{hp}")
                        for qh in range(n_qh):
                            ctx = [ps_ctx.tile([DH + 1, fd_q], F32, tag="ctx",
                                               name=f"ctx_{hp}_{qh}_{i}")
                                   for i in range(2)]
                            for kc in range(n_kc):
                                est = []
                                for hi in range(2):
                                    pss = ps_st.tile([128, fd_q], F32, tag="st")
                                    r0, r1 = (0, 64) if hi == 0 else (64, 128)
                                    for n in range(n_qn):
                                        nc.tensor.matmul(
                                            pss[:, n * 512:(n + 1) * 512],
                                            lhsT=qkT[r0:r1,
                                                     kt_off + kc * 128:
                                                     kt_off + (kc + 1) * 128
                                                     ],
                                            rhs=qkT[r0:r1,
                                                    qt_off + qh * fd_q + n * 512:
                                                    qt_off + qh * fd_q +
                                                    (n + 1) * 512],
                                            start=True, stop=True,
                                            tile_position=(r0, 0))
                                    e = pe.tile([128, fd_q], AT, tag="e")
                                    nc.scalar.activation(e[:], pss[:], EXP)
                                    msl = m01t[:, kc * s + qh * fd_q:
                                               kc * s + (qh + 1) * fd_q]
                                    if tt_idx % 3 < 2:
                                        nc.vector.tensor_tensor(
                                            e[:], e[:], msl, MULT)
                                    else:
                                        nc.gpsimd.tensor_tensor(
                                            e[:], e[:], msl, MULT)
                                    tt_idx += 1
                                    est.append(e)
                                for hi, h in enumerate((h0, h1)):
                                    for n in range(n_qn):
                                        nc.tensor.matmul(
                                            ctx[hi][:, n * 512:(n + 1) * 512],
                                            lhsT=vones[:, kc * VW + h * (DH + 1):
                                                       kc * VW +
                                                       (h + 1) * (DH + 1)
                                                       ],
                                            rhs=est[hi][:, n * 512:(n + 1) * 512
                                                        ],
                                            start=(kc == 0),
                                            stop=(kc == n_kc - 1))
                            # spill unnormalized ctx + rowsums
                            for hi, h in enumerate((h0, h1)):
                                stg = pe.tile([1, fd_q], F32R, tag="e",
                                              name=f"rstg_{hp}_{qh}_{hi}")
                                nc.scalar.copy(out=stg[:],
                                               in_=ctx[hi][DH:DH + 1, :])
                                nc.sync.dma_start(
                                    out=rs_p[hi * n_qh + qh:hi * n_qh + qh + 1, :],
                                    in_=stg[:])
                                nc.scalar.copy(
                                    out=ctxT[hi * 64:(hi + 1) * 64,
                                             hp * s + qh * fd_q:
                                             hp * s + (qh + 1) * fd_q],
                                    in_=ctx[hi][0:DH, :])

                        # normalize this pair: ctxT[c, q] *= 1/rowsum
                        with nc.allow_low_precision(
                                reason="f32r recip feeds f32r broadcast mm"):
                            nc.vector.reciprocal(rcp_p[:], rs_p[:])
                        for qh in range(n_qh):
                            r0a = pbc.tile([1, fd_q], F32R, tag="r0", bufs=1,
                                           name=f"r0_{hp}_{qh}")
                            r1a = pbc.tile([1, fd_q], F32R, tag="r1", bufs=1,
                                           name=f"r1_{hp}_{qh}")
                            nc.sync.dma_start(out=r0a[:],
                                              in_=rcp_p[qh:qh + 1, :])
                            nc.sync.dma_start(
                                out=r1a[:],
                                in_=rcp_p[n_qh + qh:n_qh + qh + 1, :])
                            bcp = ps_st.tile([128, fd_q], F32, tag="st",
                                             name=f"bcp_{hp}_{qh}")
                            for n in range(n_qn):
                                nc.tensor.matmul(
                                    bcp[:, n * 512:(n + 1) * 512],
                                    lhsT=sel_lo_t[0:1, :],
                                    rhs=r0a[0:1, n * 512:(n + 1) * 512],
                                    start=True, stop=False)
                                nc.tensor.matmul(
                                    bcp[:, n * 512:(n + 1) * 512],
                                    lhsT=sel_hi_t[0:1, :],
                                    rhs=r1a[0:1, n * 512:(n + 1) * 512],
                                    start=False, stop=True)
                            sl = ctxT[:, hp * s + qh * fd_q:
                                      hp * s + (qh + 1) * fd_q]
                            nc.vector.tensor_tensor(sl, sl, bcp[:], MULT)

                # ---------------- phase C: out projection ----------------
                with (
                    tc.tile_pool(name="poolC", bufs=2) as pc,
                    tc.tile_pool(name="poolCw", bufs=1) as pcw,
                    tc.tile_pool(name="psC", bufs=2, space="PSUM") as psC,
                ):
                    woutt = pcw.tile([128, 4 * D], F32R, tag="wout")
                    for ct in range(4):
                        nc.sync.dma_start(out=woutt[:, ct * D:(ct + 1) * D],
                                          in_=wout[ct * 128:(ct + 1) * 128, :])
                    for qc in range(n_sc):
                        for n in range(2):
                            po = psC.tile([128, 512], F32, tag="po")
                            for ct in range(4):
                                nc.tensor.matmul(
                                    po[:],
                                    lhsT=ctxT[:, ct * s + qc * 128:
                                              ct * s + (qc + 1) * 128
                                              ],
                                    rhs=woutt[:, ct * D + n * 512:
                                              ct * D + (n + 1) * 512
                                              ],
                                    start=(ct == 0), stop=(ct == 3))
                            ot = pc.tile([128, 512], F32, tag="ot")
                            nc.scalar.copy(out=ot[:], in_=po[:])
                            nc.sync.dma_start(
                                out=y[qc * 128:(qc + 1) * 128,
                                      n * 512:(n + 1) * 512],
                                in_=ot[:])

    _split_multiwait(nc)
    return nc


def _get_nc(s=S):
    key = (s, ATTN_BF16)
    if key not in _CACHE:
        _CACHE[key] = build_nc(s, attn_bf16=ATTN_BF16)
    return _CACHE[key]


ATTN_BF16 = False


def _at_np():
    if ATTN_BF16:
        import ml_dtypes
        return ml_dtypes.bfloat16
    return np.float32


def make_in_maps(x, W_qkv, b_qkv, W_out, mask, s=S):
    x = np.asarray(x, dtype=np.float32)
    W_qkv = np.asarray(W_qkv, dtype=np.float32)
    b_qkv = np.asarray(b_qkv, dtype=np.float32)
    W_out = np.asarray(W_out, dtype=np.float32)
    mask = np.asarray(mask)
    scale = 1.0 / np.sqrt(DH)
    m01 = np.ascontiguousarray((mask[0, 0] != 0).T.astype(np.uint8))
    in_maps = []
    for c in range(NCORES):
        b, g = c // 2, c % 2
        wq = W_qkv[:, g * CD:(g + 1) * CD] * scale
        wk = W_qkv[:, D + g * CD:D + (g + 1) * CD]
        in_maps.append({
            "xT": np.ascontiguousarray(x[b].T),
            "wqk": np.ascontiguousarray(np.concatenate([wq, wk], axis=1)),
            "wv": np.ascontiguousarray(
                W_qkv[:, 2 * D + g * CD:2 * D + (g + 1) * CD]),
            "bqk": np.ascontiguousarray(np.concatenate(
                [b_qkv[g * CD:(g + 1) * CD] * scale,
                 b_qkv[D + g * CD:D + (g + 1) * CD]])[None, :]),
            "bv": np.ascontiguousarray(
                b_qkv[2 * D + g * CD:2 * D + (g + 1) * CD][None, :]),
            "m01": m01,
            "wout": np.ascontiguousarray(W_out[g * CD:(g + 1) * CD, :]),
            "ones": np.ones((1, 512), dtype=np.float32),
            "ones_at": np.ones((1, 512), dtype=_at_np()),
            "sel_lo": np.concatenate(
                [np.ones(64), np.zeros(64)])[None, :].astype(np.float32),
            "sel_hi": np.concatenate(
                [np.zeros(64), np.ones(64)])[None, :].astype(np.float32),
        })
    return in_maps


def kernel(x, W_qkv, b_qkv, W_out, b_out, mask):
    from concourse.bass_utils import run_bass_kernel_spmd

    nc = _get_nc(S)
    in_maps = make_in_maps(x, W_qkv, b_qkv, W_out, mask, S)
    res = run_bass_kernel_spmd(nc, in_maps, list(range(NCORES)))
    b_out = np.asarray(b_out, dtype=np.float32)
    y = np.empty((B, S, D), dtype=np.float32)
    for b in range(B):
        y[b] = res.results[2 * b]["y"] + res.results[2 * b + 1]["y"] + b_out
    return y



# revision 5
# speedup vs baseline: 1.2053x; 1.2053x over previous
"""Multi-head attention (B=4, S=2048, D=1024, H=16) on 8 NeuronCores.

Sharding: core c -> (batch b = c//2, head-group g = c%2 of 8 heads).
Per-core: column-parallel fused qkv projection for its 8 heads,
flash-style attention (scores kept transposed: k on partitions so
softmax denominators come from a fused ones-column in the PV matmul),
row-parallel out-projection. The two partial outputs per batch are
summed on the host along with b_out.

v2: all matmul operands bf16 (host-cast, halves DMA), mask shipped as
bf16 0/1 so the DVE tensor_tensor runs in its 2-byte fast path, ACT
does exp only during attention (spills moved to Pool, phase-A psum
copies to DVE), m01 DMA issued during phase A.
"""
import sys

if "/opt/trn_rl_repo" not in sys.path:
    sys.path.insert(0, "/opt/trn_rl_repo")

import numpy as np

B, S, D, H = 4, 2048, 1024, 16
DH = D // H          # 64
HPC = H // 2         # 8 heads per core
CD = HPC * DH        # 512 local head-dims per core
NCORES = 8

_CACHE = {}


def _split_multiwait(nc):
    """walrus in this container accepts ONE sync wait per instruction;
    hoist extras onto injected same-engine EventSemaphore carriers."""
    import concourse.mybir as mybir

    for fn in nc.m.functions:
        for bb in fn.blocks:
            if not any(
                i.sync_info is not None and i.sync_info.on_wait
                and len(i.sync_info.on_wait) > 1
                for i in bb.instructions
            ):
                continue
            newlist = []
            for inst in bb.instructions:
                si = inst.sync_info
                if si is not None and si.on_wait and len(si.on_wait) > 1:
                    waits = list(si.on_wait)
                    for w in waits[:-1]:
                        ev = mybir.InstEventSemaphore(
                            name=nc.get_next_instruction_name(), ins=[], outs=[])
                        ev.engine = inst.engine
                        ev.sync_info = mybir.SyncInfo(on_wait=[w], on_update=[])
                        newlist.append(ev)
                    inst.sync_info = mybir.SyncInfo(
                        on_wait=[waits[-1]], on_update=list(si.on_update))
                newlist.append(inst)
            try:
                bb.instructions = newlist
            except Exception:
                bb.instructions.clear()
                bb.instructions.extend(newlist)


def build_nc(s=S):
    import concourse.bass as bass
    import concourse.mybir as mybir
    from concourse.tile import TileContext

    F32 = mybir.dt.float32
    F32R = mybir.dt.float32r
    BF16 = mybir.dt.bfloat16
    EXP = mybir.ActivationFunctionType.Exp
    MULT = mybir.AluOpType.mult

    n_sc = s // 128            # s-chunks of 128
    n_st = s // 512            # s-tiles of 512
    n_kc = s // 128            # k chunks (128 each)
    fd_q = min(1024, s)        # q-tile width for attention inner loop
    n_qh = s // fd_q           # q tiles
    n_qn = fd_q // 512         # 512-wide matmuls per q tile
    VW = CD + HPC              # vones row-chunk width (8 heads x 65)

    nc = bass.Bass("TRN2", num_devices=NCORES)

    xT = nc.declare_dram_parameter("xT", [D, s], BF16, isOutput=False)
    wqk = nc.declare_dram_parameter("wqk", [D, 2 * CD], BF16, isOutput=False)
    wv = nc.declare_dram_parameter("wv", [D, CD], BF16, isOutput=False)
    bqk = nc.declare_dram_parameter("bqk", [1, 2 * CD], BF16, isOutput=False)
    bv = nc.declare_dram_parameter("bv", [1, CD], BF16, isOutput=False)
    m01 = nc.declare_dram_parameter("m01", [s, s], BF16, isOutput=False)
    wout = nc.declare_dram_parameter("wout", [CD, D], BF16, isOutput=False)
    ones = nc.declare_dram_parameter("ones", [1, 512], BF16, isOutput=False)
    sel_lo = nc.declare_dram_parameter("sel_lo", [1, 128], F32R, isOutput=False)
    sel_hi = nc.declare_dram_parameter("sel_hi", [1, 128], F32R, isOutput=False)
    y = nc.declare_dram_parameter("y", [s, D], F32, isOutput=True)

    with TileContext(nc) as tc:
        with tc.tile_pool(name="persist", bufs=1) as pp:
            qkT = pp.tile([128, 8 * s], BF16, tag="qkT")       # [1024 c, s]
            vones = pp.tile([128, n_sc * VW], BF16, tag="vones")
            m01t = pp.tile([128, n_kc * s], BF16, tag="m01")
            sel_lo_t = pp.tile([1, 128], F32R, tag="sel_lo")
            sel_hi_t = pp.tile([1, 128], F32R, tag="sel_hi")
            nc.sync.dma_start(out=sel_lo_t[:], in_=sel_lo[:])
            nc.sync.dma_start(out=sel_hi_t[:], in_=sel_hi[:])

            # ---------------- phase A: qkv projection ----------------
            with tc.tile_pool(name="poolA", bufs=1) as pa, \
                 tc.tile_pool(name="psA", bufs=8, space="PSUM") as psA:
                xt = pa.tile([128, 8 * s], BF16, tag="xt")
                wqkt = pa.tile([128, 8 * 2 * CD], BF16, tag="wqkt")
                wvt = pa.tile([128, 8 * CD], BF16, tag="wvt")
                ones_row = pa.tile([1, 512], BF16, tag="ones")
                bqk_t = pa.tile([1, 2 * CD], BF16, tag="bqk")
                bv_t = pa.tile([1, CD], BF16, tag="bv")

                nc.sync.dma_start(out=ones_row[:], in_=ones[:])
                nc.sync.dma_start(out=bqk_t[:], in_=bqk[:])
                nc.sync.dma_start(out=bv_t[:], in_=bv[:])
                # x + qk weights first (feed the ct loop asap)
                for dc in range(8):
                    nc.scalar.dma_start(
                        out=wqkt[:, dc * 2 * CD:(dc + 1) * 2 * CD],
                        in_=wqk[dc * 128:(dc + 1) * 128, :])
                    nc.sync.dma_start(out=xt[:, dc * s:(dc + 1) * s],
                                      in_=xT[dc * 128:(dc + 1) * 128, :])
                for dc in range(8):
                    nc.scalar.dma_start(out=wvt[:, dc * CD:(dc + 1) * CD],
                                        in_=wv[dc * 128:(dc + 1) * 128, :])
                # mask: needed only at attention start; queue behind x
                for kc in range(n_kc):
                    nc.sync.dma_start(out=m01t[:, kc * s:(kc + 1) * s],
                                      in_=m01[kc * 128:(kc + 1) * 128, :])
                # ones columns of vones (the rest is overwritten below)
                vones_cols = vones[:].rearrange(
                    "p (ch e) -> p ch e", e=DH + 1)[:, :, DH:DH + 1]
                ones_rep = bass.AP(ones, 0,
                                   [[0, 128], [1, n_sc * HPC], [1, 1]])
                nc.sync.dma_start(out=vones_cols, in_=ones_rep)

                # q/k: qkT[c, :] = (W.T x.T), c-tiles of 128
                for ct in range(8):
                    pst = [psA.tile([128, 512], F32, tag="pa",
                                    name=f"psqk_{ct}_{st}")
                           for st in range(n_st)]
                    for dc in range(8):
                        wsl = wqkt[:, dc * 2 * CD + ct * 128:
                                   dc * 2 * CD + (ct + 1) * 128]
                        for st in range(n_st):
                            nc.tensor.matmul(
                                pst[st][:],
                                lhsT=wsl,
                                rhs=xt[:, dc * s + st * 512:
                                       dc * s + (st + 1) * 512],
                                start=(dc == 0), stop=False)
                    for st in range(n_st):
                        nc.tensor.matmul(
                            pst[st][:],
                            lhsT=bqk_t[0:1, ct * 128:(ct + 1) * 128],
                            rhs=ones_row[0:1, :],
                            start=False, stop=True)
                        nc.scalar.copy(
                            out=qkT[:, ct * s + st * 512:ct * s + (st + 1) * 512],
                            in_=pst[st][:])

                # v: natural [s, c] layout, s-chunks of 128, fused ones col
                for scg in range(n_sc // 4):
                    psv = [psA.tile([128, 512], F32, tag="pa",
                                    name=f"psv_{scg}_{i}")
                           for i in range(4)]
                    for dc in range(8):
                        for sci in range(4):
                            sc = scg * 4 + sci
                            nc.tensor.matmul(
                                psv[sci][:],
                                lhsT=xt[:, dc * s + sc * 128:
                                        dc * s + (sc + 1) * 128],
                                rhs=wvt[:, dc * CD:(dc + 1) * CD],
                                start=(dc == 0), stop=False)
                    for sci in range(4):
                        sc = scg * 4 + sci
                        nc.tensor.matmul(
                            psv[sci][:],
                            lhsT=ones_row[0:1, 0:128],
                            rhs=bv_t[0:1, :],
                            start=False, stop=True)
                        dst = vones[:, sc * VW:(sc + 1) * VW].rearrange(
                            "p (h e) -> p h e", e=DH + 1)[:, :, 0:DH]
                        src = psv[sci][:].rearrange("p (h e) -> p h e", e=DH)
                        nc.vector.tensor_copy(dst, src)

            # ---------------- phase B: attention ----------------
            with tc.tile_pool(name="poolB", bufs=1) as pb:
                ctxT = pb.tile([128, 4 * s], BF16, tag="ctxT")   # [512 c, s]
                with (
                    tc.tile_pool(name="poolE", bufs=4) as pe,
                    tc.tile_pool(name="poolBc", bufs=2) as pbc,
                    tc.tile_pool(name="psB_st", bufs=2, space="PSUM") as ps_st,
                    tc.tile_pool(name="psB_ctx", bufs=2, space="PSUM") as ps_ctx,
                ):
                    for hp in range(4):
                        h0, h1 = 2 * hp, 2 * hp + 1
                        kt_off = (4 + hp) * s   # K pair c-tile offset in qkT
                        qt_off = hp * s         # Q pair c-tile offset
                        rs_p = pbc.tile([2 * n_qh, fd_q], F32R, tag="rsp",
                                        name=f"rs_{hp}")
                        rcp_p = pbc.tile([2 * n_qh, fd_q], F32R, tag="rcpp",
                                         name=f"rcp_{hp}")
                        for qh in range(n_qh):
                            ctx = [ps_ctx.tile([DH + 1, fd_q], F32, tag="ctx",
                                               name=f"ctx_{hp}_{qh}_{i}")
                                   for i in range(2)]
                            for kc in range(n_kc):
                                est = []
                                for hi in range(2):
                                    pss = ps_st.tile([128, fd_q], F32, tag="st")
                                    r0, r1 = (0, 64) if hi == 0 else (64, 128)
                                    for n in range(n_qn):
                                        nc.tensor.matmul(
                                            pss[:, n * 512:(n + 1) * 512],
                                            lhsT=qkT[r0:r1,
                                                     kt_off + kc * 128:
                                                     kt_off + (kc + 1) * 128
                                                     ],
                                            rhs=qkT[r0:r1,
                                                    qt_off + qh * fd_q + n * 512:
                                                    qt_off + qh * fd_q +
                                                    (n + 1) * 512],
                                            start=True, stop=True,
                                            tile_position=(r0, 0))
                                    e = pe.tile([128, fd_q], BF16, tag="e")
                                    nc.scalar.activation(e[:], pss[:], EXP)
                                    msl = m01t[:, kc * s + qh * fd_q:
                                               kc * s + (qh + 1) * fd_q]
                                    nc.vector.tensor_tensor(
                                        e[:], e[:], msl, MULT)
                                    est.append(e)
                                for hi, h in enumerate((h0, h1)):
                                    for n in range(n_qn):
                                        nc.tensor.matmul(
                                            ctx[hi][:, n * 512:(n + 1) * 512],
                                            lhsT=vones[:, kc * VW + h * (DH + 1):
                                                       kc * VW +
                                                       (h + 1) * (DH + 1)
                                                       ],
                                            rhs=est[hi][:, n * 512:(n + 1) * 512
                                                        ],
                                            start=(kc == 0),
                                            stop=(kc == n_kc - 1))
                            # spill unnormalized ctx + rowsums
                            for hi, h in enumerate((h0, h1)):
                                stg = pbc.tile([1, fd_q], F32R, tag="stg",
                                               name=f"rstg_{hp}_{qh}_{hi}")
                                nc.vector.tensor_copy(stg[:],
                                                      ctx[hi][DH:DH + 1, :])
                                nc.sync.dma_start(
                                    out=rs_p[hi * n_qh + qh:hi * n_qh + qh + 1, :],
                                    in_=stg[:])
                                nc.vector.tensor_copy(
                                    ctxT[hi * 64:(hi + 1) * 64,
                                         hp * s + qh * fd_q:
                                         hp * s + (qh + 1) * fd_q],
                                    ctx[hi][0:DH, :])

                        # normalize this pair: ctxT[c, q] *= 1/rowsum
                        with nc.allow_low_precision(
                                reason="f32r recip feeds f32r broadcast mm"):
                            nc.vector.reciprocal(rcp_p[:], rs_p[:])
                        for qh in range(n_qh):
                            r0a = pbc.tile([1, fd_q], F32R, tag="r0", bufs=1,
                                           name=f"r0_{hp}_{qh}")
                            r1a = pbc.tile([1, fd_q], F32R, tag="r1", bufs=1,
                                           name=f"r1_{hp}_{qh}")
                            nc.sync.dma_start(out=r0a[:],
                                              in_=rcp_p[qh:qh + 1, :])
                            nc.sync.dma_start(
                                out=r1a[:],
                                in_=rcp_p[n_qh + qh:n_qh + qh + 1, :])
                            bcp = ps_st.tile([128, fd_q], F32, tag="st",
                                             name=f"bcp_{hp}_{qh}")
                            for n in range(n_qn):
                                nc.tensor.matmul(
                                    bcp[:, n * 512:(n + 1) * 512],
                                    lhsT=sel_lo_t[0:1, :],
                                    rhs=r0a[0:1, n * 512:(n + 1) * 512],
                                    start=True, stop=False)
                                nc.tensor.matmul(
                                    bcp[:, n * 512:(n + 1) * 512],
                                    lhsT=sel_hi_t[0:1, :],
                                    rhs=r1a[0:1, n * 512:(n + 1) * 512],
                                    start=False, stop=True)
                            sl = ctxT[:, hp * s + qh * fd_q:
                                      hp * s + (qh + 1) * fd_q]
                            nc.vector.tensor_tensor(sl, sl, bcp[:], MULT)

                # ---------------- phase C: out projection ----------------
                with (
                    tc.tile_pool(name="poolC", bufs=2) as pc,
                    tc.tile_pool(name="poolCw", bufs=1) as pcw,
                    tc.tile_pool(name="psC", bufs=2, space="PSUM") as psC,
                ):
                    woutt = pcw.tile([128, 4 * D], BF16, tag="wout")
                    for ct in range(4):
                        nc.sync.dma_start(out=woutt[:, ct * D:(ct + 1) * D],
                                          in_=wout[ct * 128:(ct + 1) * 128, :])
                    for qc in range(n_sc):
                        for n in range(2):
                            po = psC.tile([128, 512], F32, tag="po")
                            for ct in range(4):
                                nc.tensor.matmul(
                                    po[:],
                                    lhsT=ctxT[:, ct * s + qc * 128:
                                              ct * s + (qc + 1) * 128
                                              ],
                                    rhs=woutt[:, ct * D + n * 512:
                                              ct * D + (n + 1) * 512
                                              ],
                                    start=(ct == 0), stop=(ct == 3))
                            ot = pc.tile([128, 512], F32, tag="ot")
                            nc.scalar.copy(out=ot[:], in_=po[:])
                            nc.sync.dma_start(
                                out=y[qc * 128:(qc + 1) * 128,
                                      n * 512:(n + 1) * 512],
                                in_=ot[:])

    _split_multiwait(nc)
    return nc


def _get_nc(s=S):
    if s not in _CACHE:
        _CACHE[s] = build_nc(s)
    return _CACHE[s]


def make_in_maps(x, W_qkv, b_qkv, W_out, mask, s=S):
    import ml_dtypes

    BF = ml_dtypes.bfloat16
    x = np.asarray(x, dtype=np.float32)
    W_qkv = np.asarray(W_qkv, dtype=np.float32)
    b_qkv = np.asarray(b_qkv, dtype=np.float32)
    W_out = np.asarray(W_out, dtype=np.float32)
    mask = np.asarray(mask)
    scale = 1.0 / np.sqrt(DH)
    m01 = np.ascontiguousarray((mask[0, 0] != 0).T.astype(BF))
    in_maps = []
    for c in range(NCORES):
        b, g = c // 2, c % 2
        wq = W_qkv[:, g * CD:(g + 1) * CD] * scale
        wk = W_qkv[:, D + g * CD:D + (g + 1) * CD]
        in_maps.append({
            "xT": np.ascontiguousarray(x[b].T.astype(BF)),
            "wqk": np.ascontiguousarray(
                np.concatenate([wq, wk], axis=1).astype(BF)),
            "wv": np.ascontiguousarray(
                W_qkv[:, 2 * D + g * CD:2 * D + (g + 1) * CD].astype(BF)),
            "bqk": np.ascontiguousarray(np.concatenate(
                [b_qkv[g * CD:(g + 1) * CD] * scale,
                 b_qkv[D + g * CD:D + (g + 1) * CD]])[None, :].astype(BF)),
            "bv": np.ascontiguousarray(
                b_qkv[2 * D + g * CD:2 * D + (g + 1) * CD][None, :].astype(BF)),
            "m01": m01,
            "wout": np.ascontiguousarray(
                W_out[g * CD:(g + 1) * CD, :].astype(BF)),
            "ones": np.ones((1, 512), dtype=BF),
            "sel_lo": np.concatenate(
                [np.ones(64), np.zeros(64)])[None, :].astype(np.float32),
            "sel_hi": np.concatenate(
                [np.zeros(64), np.ones(64)])[None, :].astype(np.float32),
        })
    return in_maps


def kernel(x, W_qkv, b_qkv, W_out, b_out, mask):
    from concourse.bass_utils import run_bass_kernel_spmd

    nc = _get_nc(S)
    in_maps = make_in_maps(x, W_qkv, b_qkv, W_out, mask, S)
    res = run_bass_kernel_spmd(nc, in_maps, list(range(NCORES)))
    b_out = np.asarray(b_out, dtype=np.float32)
    y = np.empty((B, S, D), dtype=np.float32)
    for b in range(B):
        y[b] = res.results[2 * b]["y"] + res.results[2 * b + 1]["y"] + b_out
    return y


# revision 7
# speedup vs baseline: 1.3756x; 1.1413x over previous
"""Multi-head attention (B=4, S=2048, D=1024, H=16) on 8 NeuronCores.

Sharding: core c -> (batch b = c//2, head-group g = c%2 of 8 heads).
Per-core: column-parallel fused qkv projection for its 8 heads,
flash-style attention (scores kept transposed: k on partitions so
softmax denominators come from a fused ones-column in the PV matmul),
row-parallel out-projection. The two partial outputs per batch are
summed on the host along with b_out.

v2: all matmul operands bf16 (host-cast, halves DMA), mask shipped as
bf16 0/1 so the DVE tensor_tensor runs in its 2-byte fast path, ACT
does exp only during attention (spills moved to Pool, phase-A psum
copies to DVE), m01 DMA issued during phase A.
"""
import sys

if "/opt/trn_rl_repo" not in sys.path:
    sys.path.insert(0, "/opt/trn_rl_repo")

import numpy as np

B, S, D, H = 4, 2048, 1024, 16
DH = D // H          # 64
HPC = H // 2         # 8 heads per core
CD = HPC * DH        # 512 local head-dims per core
NCORES = 8

_CACHE = {}


def _split_multiwait(nc):
    """walrus in this container accepts ONE sync wait per instruction;
    hoist extras onto injected same-engine EventSemaphore carriers."""
    import concourse.mybir as mybir

    for fn in nc.m.functions:
        for bb in fn.blocks:
            if not any(
                i.sync_info is not None and i.sync_info.on_wait
                and len(i.sync_info.on_wait) > 1
                for i in bb.instructions
            ):
                continue
            newlist = []
            for inst in bb.instructions:
                si = inst.sync_info
                if si is not None and si.on_wait and len(si.on_wait) > 1:
                    waits = list(si.on_wait)
                    for w in waits[:-1]:
                        ev = mybir.InstEventSemaphore(
                            name=nc.get_next_instruction_name(), ins=[], outs=[])
                        ev.engine = inst.engine
                        ev.sync_info = mybir.SyncInfo(on_wait=[w], on_update=[])
                        newlist.append(ev)
                    inst.sync_info = mybir.SyncInfo(
                        on_wait=[waits[-1]], on_update=list(si.on_update))
                newlist.append(inst)
            try:
                bb.instructions = newlist
            except Exception:
                bb.instructions.clear()
                bb.instructions.extend(newlist)


def build_nc(s=S):
    import concourse.bass as bass
    import concourse.mybir as mybir
    from concourse.tile import TileContext

    F32 = mybir.dt.float32
    F32R = mybir.dt.float32r
    BF16 = mybir.dt.bfloat16
    EXP = mybir.ActivationFunctionType.Exp
    MULT = mybir.AluOpType.mult

    n_sc = s // 128            # s-chunks of 128
    n_st = s // 512            # s-tiles of 512
    n_kc = s // 128            # k chunks (128 each)
    fd_q = min(1024, s)        # q-tile width for attention inner loop
    n_qh = s // fd_q           # q tiles
    n_qn = fd_q // 512         # 512-wide matmuls per q tile
    VW = CD + HPC              # vones row-chunk width (8 heads x 65)

    nc = bass.Bass("TRN2", num_devices=NCORES)

    xT = nc.declare_dram_parameter("xT", [D, s], BF16, isOutput=False)
    wqk = nc.declare_dram_parameter("wqk", [D, 2 * CD], BF16, isOutput=False)
    wv = nc.declare_dram_parameter("wv", [D, CD], BF16, isOutput=False)
    bqk = nc.declare_dram_parameter("bqk", [1, 2 * CD], BF16, isOutput=False)
    bv = nc.declare_dram_parameter("bv", [1, CD], BF16, isOutput=False)
    m01 = nc.declare_dram_parameter("m01", [s, s], BF16, isOutput=False)
    wout = nc.declare_dram_parameter("wout", [CD, D], BF16, isOutput=False)
    ones = nc.declare_dram_parameter("ones", [1, 512], BF16, isOutput=False)
    sel_lo = nc.declare_dram_parameter("sel_lo", [1, 128], F32R, isOutput=False)
    sel_hi = nc.declare_dram_parameter("sel_hi", [1, 128], F32R, isOutput=False)
    y = nc.declare_dram_parameter("y", [s, D], F32, isOutput=True)

    with TileContext(nc) as tc:
        with tc.tile_pool(name="persist", bufs=1) as pp:
            qkT = pp.tile([128, 8 * s], BF16, tag="qkT")       # [1024 c, s]
            vones = pp.tile([128, n_sc * VW], BF16, tag="vones")
            m01t = pp.tile([128, n_kc * s], BF16, tag="m01")
            sel_lo_t = pp.tile([1, 128], F32R, tag="sel_lo")
            sel_hi_t = pp.tile([1, 128], F32R, tag="sel_hi")
            nc.sync.dma_start(out=sel_lo_t[:], in_=sel_lo[:])
            nc.sync.dma_start(out=sel_hi_t[:], in_=sel_hi[:])

            # ---------------- phase A: qkv projection ----------------
            with tc.tile_pool(name="poolA", bufs=1) as pa, \
                 tc.tile_pool(name="psA", bufs=8, space="PSUM") as psA:
                xt = pa.tile([128, 8 * s], BF16, tag="xt")
                wqkt = pa.tile([128, 8 * 2 * CD], BF16, tag="wqkt")
                wvt = pa.tile([128, 8 * CD], BF16, tag="wvt")
                ones_row = pa.tile([1, 512], BF16, tag="ones")
                bqk_t = pa.tile([1, 2 * CD], BF16, tag="bqk")
                bv_t = pa.tile([1, CD], BF16, tag="bv")

                nc.sync.dma_start(out=ones_row[:], in_=ones[:])
                nc.sync.dma_start(out=bqk_t[:], in_=bqk[:])
                nc.sync.dma_start(out=bv_t[:], in_=bv[:])
                # x + qk weights first (feed the ct loop asap)
                for dc in range(8):
                    nc.scalar.dma_start(
                        out=wqkt[:, dc * 2 * CD:(dc + 1) * 2 * CD],
                        in_=wqk[dc * 128:(dc + 1) * 128, :])
                    nc.sync.dma_start(out=xt[:, dc * s:(dc + 1) * s],
                                      in_=xT[dc * 128:(dc + 1) * 128, :])
                for dc in range(8):
                    nc.scalar.dma_start(out=wvt[:, dc * CD:(dc + 1) * CD],
                                        in_=wv[dc * 128:(dc + 1) * 128, :])
                # mask: needed only at attention start; queue behind x,
                # split across both hwdge queues
                for kc in range(n_kc):
                    eng = nc.sync if kc % 2 == 0 else nc.scalar
                    eng.dma_start(out=m01t[:, kc * s:(kc + 1) * s],
                                  in_=m01[kc * 128:(kc + 1) * 128, :])
                # ones columns of vones (the rest is overwritten below)
                vones_cols = vones[:].rearrange(
                    "p (ch e) -> p ch e", e=DH + 1)[:, :, DH:DH + 1]
                nc.gpsimd.memset(vones_cols, 1.0)

                # q/k: qkT[c, :] = (W.T x.T), c-tiles of 128
                for ct in range(8):
                    pst = [psA.tile([128, 512], F32, tag="pa",
                                    name=f"psqk_{ct}_{st}")
                           for st in range(n_st)]
                    for dc in range(8):
                        wsl = wqkt[:, dc * 2 * CD + ct * 128:
                                   dc * 2 * CD + (ct + 1) * 128]
                        for st in range(n_st):
                            nc.tensor.matmul(
                                pst[st][:],
                                lhsT=wsl,
                                rhs=xt[:, dc * s + st * 512:
                                       dc * s + (st + 1) * 512],
                                start=(dc == 0), stop=False)
                    for st in range(n_st):
                        nc.tensor.matmul(
                            pst[st][:],
                            lhsT=bqk_t[0:1, ct * 128:(ct + 1) * 128],
                            rhs=ones_row[0:1, :],
                            start=False, stop=True)
                        nc.scalar.copy(
                            out=qkT[:, ct * s + st * 512:ct * s + (st + 1) * 512],
                            in_=pst[st][:])

                # v: natural [s, c] layout, s-chunks of 128, fused ones col
                for scg in range(n_sc // 4):
                    psv = [psA.tile([128, 512], F32, tag="pa",
                                    name=f"psv_{scg}_{i}")
                           for i in range(4)]
                    for dc in range(8):
                        for sci in range(4):
                            sc = scg * 4 + sci
                            nc.tensor.matmul(
                                psv[sci][:],
                                lhsT=xt[:, dc * s + sc * 128:
                                        dc * s + (sc + 1) * 128],
                                rhs=wvt[:, dc * CD:(dc + 1) * CD],
                                start=(dc == 0), stop=False)
                    for sci in range(4):
                        sc = scg * 4 + sci
                        nc.tensor.matmul(
                            psv[sci][:],
                            lhsT=ones_row[0:1, 0:128],
                            rhs=bv_t[0:1, :],
                            start=False, stop=True)
                        dst = vones[:, sc * VW:(sc + 1) * VW].rearrange(
                            "p (h e) -> p h e", e=DH + 1)[:, :, 0:DH]
                        src = psv[sci][:].rearrange("p (h e) -> p h e", e=DH)
                        nc.vector.tensor_copy(dst, src)

            # ---------------- phase B: attention ----------------
            with tc.tile_pool(name="poolB", bufs=1) as pb:
                ctxT = pb.tile([128, 4 * s], BF16, tag="ctxT")   # [512 c, s]
                with (
                    tc.tile_pool(name="poolE", bufs=4) as pe,
                    tc.tile_pool(name="poolBc", bufs=2) as pbc,
                    tc.tile_pool(name="psB_st", bufs=2, space="PSUM") as ps_st,
                    tc.tile_pool(name="psB_ctx", bufs=2, space="PSUM") as ps_ctx,
                ):
                    for hp in range(4):
                        h0, h1 = 2 * hp, 2 * hp + 1
                        kt_off = (4 + hp) * s   # K pair c-tile offset in qkT
                        qt_off = hp * s         # Q pair c-tile offset
                        rs_p = pbc.tile([2 * n_qh, fd_q], F32R, tag="rsp",
                                        name=f"rs_{hp}")
                        rcp_p = pbc.tile([2 * n_qh, fd_q], F32R, tag="rcpp",
                                         name=f"rcp_{hp}")
                        for qh in range(n_qh):
                            ctx = [ps_ctx.tile([DH + 1, fd_q], F32, tag="ctx",
                                               name=f"ctx_{hp}_{qh}_{i}")
                                   for i in range(2)]
                            for kc in range(n_kc):
                                est = []
                                for hi in range(2):
                                    pss = ps_st.tile([128, fd_q], F32, tag="st")
                                    r0, r1 = (0, 64) if hi == 0 else (64, 128)
                                    for n in range(n_qn):
                                        nc.tensor.matmul(
                                            pss[:, n * 512:(n + 1) * 512],
                                            lhsT=qkT[r0:r1,
                                                     kt_off + kc * 128:
                                                     kt_off + (kc + 1) * 128
                                                     ],
                                            rhs=qkT[r0:r1,
                                                    qt_off + qh * fd_q + n * 512:
                                                    qt_off + qh * fd_q +
                                                    (n + 1) * 512],
                                            start=True, stop=True,
                                            tile_position=(r0, 0))
                                    e = pe.tile([128, fd_q], BF16, tag="e")
                                    nc.scalar.activation(e[:], pss[:], EXP)
                                    msl = m01t[:, kc * s + qh * fd_q:
                                               kc * s + (qh + 1) * fd_q]
                                    nc.vector.tensor_tensor(
                                        e[:], e[:], msl, MULT)
                                    est.append(e)
                                for hi, h in enumerate((h0, h1)):
                                    for n in range(n_qn):
                                        nc.tensor.matmul(
                                            ctx[hi][:, n * 512:(n + 1) * 512],
                                            lhsT=vones[:, kc * VW + h * (DH + 1):
                                                       kc * VW +
                                                       (h + 1) * (DH + 1)
                                                       ],
                                            rhs=est[hi][:, n * 512:(n + 1) * 512
                                                        ],
                                            start=(kc == 0),
                                            stop=(kc == n_kc - 1))
                            # spill unnormalized ctx + rowsums
                            for hi, h in enumerate((h0, h1)):
                                stg = pbc.tile([1, fd_q], F32R, tag="stg",
                                               name=f"rstg_{hp}_{qh}_{hi}")
                                nc.vector.tensor_copy(stg[:],
                                                      ctx[hi][DH:DH + 1, :])
                                nc.sync.dma_start(
                                    out=rs_p[hi * n_qh + qh:hi * n_qh + qh + 1, :],
                                    in_=stg[:])
                                nc.vector.tensor_copy(
                                    ctxT[hi * 64:(hi + 1) * 64,
                                         hp * s + qh * fd_q:
                                         hp * s + (qh + 1) * fd_q],
                                    ctx[hi][0:DH, :])

                        # normalize this pair: ctxT[c, q] *= 1/rowsum
                        with nc.allow_low_precision(
                                reason="f32r recip feeds f32r broadcast mm"):
                            nc.vector.reciprocal(rcp_p[:], rs_p[:])
                        for qh in range(n_qh):
                            r0a = pbc.tile([1, fd_q], F32R, tag="r0", bufs=1,
                                           name=f"r0_{hp}_{qh}")
                            r1a = pbc.tile([1, fd_q], F32R, tag="r1", bufs=1,
                                           name=f"r1_{hp}_{qh}")
                            nc.sync.dma_start(out=r0a[:],
                                              in_=rcp_p[qh:qh + 1, :])
                            nc.sync.dma_start(
                                out=r1a[:],
                                in_=rcp_p[n_qh + qh:n_qh + qh + 1, :])
                            bcp = ps_st.tile([128, fd_q], F32, tag="st",
                                             name=f"bcp_{hp}_{qh}")
                            for n in range(n_qn):
                                nc.tensor.matmul(
                                    bcp[:, n * 512:(n + 1) * 512],
                                    lhsT=sel_lo_t[0:1, :],
                                    rhs=r0a[0:1, n * 512:(n + 1) * 512],
                                    start=True, stop=False)
                                nc.tensor.matmul(
                                    bcp[:, n * 512:(n + 1) * 512],
                                    lhsT=sel_hi_t[0:1, :],
                                    rhs=r1a[0:1, n * 512:(n + 1) * 512],
                                    start=False, stop=True)
                            sl = ctxT[:, hp * s + qh * fd_q:
                                      hp * s + (qh + 1) * fd_q]
                            nc.vector.tensor_tensor(sl, sl, bcp[:], MULT)

                # ---------------- phase C: out projection ----------------
                with (
                    tc.tile_pool(name="poolC", bufs=2) as pc,
                    tc.tile_pool(name="poolCw", bufs=1) as pcw,
                    tc.tile_pool(name="psC", bufs=2, space="PSUM") as psC,
                ):
                    woutt = pcw.tile([128, 4 * D], BF16, tag="wout")
                    for ct in range(4):
                        nc.sync.dma_start(out=woutt[:, ct * D:(ct + 1) * D],
                                          in_=wout[ct * 128:(ct + 1) * 128, :])
                    for qc in range(n_sc):
                        ot = pc.tile([128, D], F32, tag="ot")
                        for n in range(2):
                            po = psC.tile([128, 512], F32, tag="po")
                            for ct in range(4):
                                nc.tensor.matmul(
                                    po[:],
                                    lhsT=ctxT[:, ct * s + qc * 128:
                                              ct * s + (qc + 1) * 128
                                              ],
                                    rhs=woutt[:, ct * D + n * 512:
                                              ct * D + (n + 1) * 512
                                              ],
                                    start=(ct == 0), stop=(ct == 3))
                            nc.scalar.copy(out=ot[:, n * 512:(n + 1) * 512],
                                           in_=po[:])
                        nc.sync.dma_start(
                            out=y[qc * 128:(qc + 1) * 128, :], in_=ot[:])

    _split_multiwait(nc)
    return nc


def _get_nc(s=S):
    if s not in _CACHE:
        _CACHE[s] = build_nc(s)
    return _CACHE[s]


def make_in_maps(x, W_qkv, b_qkv, W_out, mask, s=S):
    import ml_dtypes

    BF = ml_dtypes.bfloat16
    x = np.asarray(x, dtype=np.float32)
    W_qkv = np.asarray(W_qkv, dtype=np.float32)
    b_qkv = np.asarray(b_qkv, dtype=np.float32)
    W_out = np.asarray(W_out, dtype=np.float32)
    mask = np.asarray(mask)
    scale = 1.0 / np.sqrt(DH)
    m01 = np.ascontiguousarray((mask[0, 0] != 0).T.astype(BF))
    in_maps = []
    for c in range(NCORES):
        b, g = c // 2, c % 2
        wq = W_qkv[:, g * CD:(g + 1) * CD] * scale
        wk = W_qkv[:, D + g * CD:D + (g + 1) * CD]
        in_maps.append({
            "xT": np.ascontiguousarray(x[b].T.astype(BF)),
            "wqk": np.ascontiguousarray(
                np.concatenate([wq, wk], axis=1).astype(BF)),
            "wv": np.ascontiguousarray(
                W_qkv[:, 2 * D + g * CD:2 * D + (g + 1) * CD].astype(BF)),
            "bqk": np.ascontiguousarray(np.concatenate(
                [b_qkv[g * CD:(g + 1) * CD] * scale,
                 b_qkv[D + g * CD:D + (g + 1) * CD]])[None, :].astype(BF)),
            "bv": np.ascontiguousarray(
                b_qkv[2 * D + g * CD:2 * D + (g + 1) * CD][None, :].astype(BF)),
            "m01": m01,
            "wout": np.ascontiguousarray(
                W_out[g * CD:(g + 1) * CD, :].astype(BF)),
            "ones": np.ones((1, 512), dtype=BF),
            "sel_lo": np.concatenate(
                [np.ones(64), np.zeros(64)])[None, :].astype(np.float32),
            "sel_hi": np.concatenate(
                [np.zeros(64), np.ones(64)])[None, :].astype(np.float32),
        })
    return in_maps


def kernel(x, W_qkv, b_qkv, W_out, b_out, mask):
    from concourse.bass_utils import run_bass_kernel_spmd

    nc = _get_nc(S)
    in_maps = make_in_maps(x, W_qkv, b_qkv, W_out, mask, S)
    res = run_bass_kernel_spmd(nc, in_maps, list(range(NCORES)))
    b_out = np.asarray(b_out, dtype=np.float32)
    y = np.empty((B, S, D), dtype=np.float32)
    for b in range(B):
        y[b] = res.results[2 * b]["y"] + res.results[2 * b + 1]["y"] + b_out
    return y


# revision 16
# speedup vs baseline: 1.4082x; 1.0237x over previous
"""Multi-head attention (B=4, S=2048, D=1024, H=16) on 8 NeuronCores.

Sharding: core c -> (batch b = c//2, head-group g = c%2 of 8 heads).
Per-core: column-parallel fused qkv projection for its 8 heads,
flash-style attention (scores kept transposed: k on partitions so
softmax denominators come from a fused ones-column in the PV matmul),
row-parallel out-projection. The two partial outputs per batch are
summed on the host along with b_out.

v2: all matmul operands bf16 (host-cast, halves DMA), mask shipped as
bf16 0/1 so the DVE tensor_tensor runs in its 2-byte fast path, ACT
does exp only during attention (spills moved to Pool, phase-A psum
copies to DVE), m01 DMA issued during phase A.
"""
import sys

if "/opt/trn_rl_repo" not in sys.path:
    sys.path.insert(0, "/opt/trn_rl_repo")

import numpy as np

B, S, D, H = 4, 2048, 1024, 16
DH = D // H          # 64
HPC = H // 2         # 8 heads per core
CD = HPC * DH        # 512 local head-dims per core
NCORES = 8

_CACHE = {}


def _split_multiwait(nc):
    """walrus in this container accepts ONE sync wait per instruction;
    hoist extras onto injected same-engine EventSemaphore carriers."""
    import concourse.mybir as mybir

    for fn in nc.m.functions:
        for bb in fn.blocks:
            if not any(
                i.sync_info is not None and i.sync_info.on_wait
                and len(i.sync_info.on_wait) > 1
                for i in bb.instructions
            ):
                continue
            newlist = []
            for inst in bb.instructions:
                si = inst.sync_info
                if si is not None and si.on_wait and len(si.on_wait) > 1:
                    waits = list(si.on_wait)
                    for w in waits[:-1]:
                        ev = mybir.InstEventSemaphore(
                            name=nc.get_next_instruction_name(), ins=[], outs=[])
                        ev.engine = inst.engine
                        ev.sync_info = mybir.SyncInfo(on_wait=[w], on_update=[])
                        newlist.append(ev)
                    inst.sync_info = mybir.SyncInfo(
                        on_wait=[waits[-1]], on_update=list(si.on_update))
                newlist.append(inst)
            try:
                bb.instructions = newlist
            except Exception:
                bb.instructions.clear()
                bb.instructions.extend(newlist)


def build_nc(s=S):
    import concourse.bass as bass
    import concourse.mybir as mybir
    from concourse.tile import TileContext

    F32 = mybir.dt.float32
    F32R = mybir.dt.float32r
    BF16 = mybir.dt.bfloat16
    EXP = mybir.ActivationFunctionType.Exp
    MULT = mybir.AluOpType.mult

    n_sc = s // 128            # s-chunks of 128
    n_st = s // 512            # s-tiles of 512
    n_kc = s // 128            # k chunks (128 each)
    fd_q = min(1024, s)        # q-tile width for attention inner loop
    n_qh = s // fd_q           # q tiles
    n_qn = fd_q // 512         # 512-wide matmuls per q tile
    VW = CD + HPC              # vones row-chunk width (8 heads x 65)

    nc = bass.Bass("TRN2", num_devices=NCORES)

    xT = nc.declare_dram_parameter("xT", [D, s], BF16, isOutput=False)
    wqk = nc.declare_dram_parameter("wqk", [D, 2 * CD], BF16, isOutput=False)
    wv = nc.declare_dram_parameter("wv", [D, CD], BF16, isOutput=False)
    bqk = nc.declare_dram_parameter("bqk", [1, 2 * CD], BF16, isOutput=False)
    bv = nc.declare_dram_parameter("bv", [1, CD], BF16, isOutput=False)
    m01 = nc.declare_dram_parameter("m01", [s, s], BF16, isOutput=False)
    wout = nc.declare_dram_parameter("wout", [CD, D], BF16, isOutput=False)
    ones = nc.declare_dram_parameter("ones", [1, 512], BF16, isOutput=False)
    y = nc.declare_dram_parameter("y", [s, D], F32, isOutput=True)

    with TileContext(nc) as tc:
        with tc.tile_pool(name="persist", bufs=1) as pp:
            qkT = pp.tile([128, 8 * s], BF16, tag="qkT")       # [1024 c, s]
            vones = pp.tile([128, n_sc * VW], BF16, tag="vones")
            m01t = pp.tile([128, n_kc * s], BF16, tag="m01")

            # ---------------- phase A: qkv projection ----------------
            with tc.tile_pool(name="poolA", bufs=1) as pa, \
                 tc.tile_pool(name="psA", bufs=8, space="PSUM") as psA:
                xt = pa.tile([128, 8 * s], BF16, tag="xt")
                wqkt = pa.tile([128, 8 * 2 * CD], BF16, tag="wqkt")
                wvt = pa.tile([128, 8 * CD], BF16, tag="wvt")
                ones_row = pa.tile([1, 512], BF16, tag="ones")
                bqk_t = pa.tile([1, 2 * CD], BF16, tag="bqk")
                bv_t = pa.tile([1, CD], BF16, tag="bv")

                nc.sync.dma_start(out=ones_row[:], in_=ones[:])
                nc.sync.dma_start(out=bqk_t[:], in_=bqk[:])
                nc.sync.dma_start(out=bv_t[:], in_=bv[:])
                # x + qk weights first (feed the ct loop asap)
                for dc in range(8):
                    nc.scalar.dma_start(
                        out=wqkt[:, dc * 2 * CD:(dc + 1) * 2 * CD],
                        in_=wqk[dc * 128:(dc + 1) * 128, :])
                    nc.sync.dma_start(out=xt[:, dc * s:(dc + 1) * s],
                                      in_=xT[dc * 128:(dc + 1) * 128, :])
                for dc in range(8):
                    nc.scalar.dma_start(out=wvt[:, dc * CD:(dc + 1) * CD],
                                        in_=wv[dc * 128:(dc + 1) * 128, :])
                # mask: needed only at attention start; queue behind x,
                # split across both hwdge queues
                for kc in range(n_kc):
                    eng = nc.sync if kc % 2 == 0 else nc.scalar
                    eng.dma_start(out=m01t[:, kc * s:(kc + 1) * s],
                                  in_=m01[kc * 128:(kc + 1) * 128, :])
                # ones columns of vones (the rest is overwritten below)
                vones_cols = vones[:].rearrange(
                    "p (ch e) -> p ch e", e=DH + 1)[:, :, DH:DH + 1]
                nc.gpsimd.memset(vones_cols, 1.0)

                # q/k: qkT[c, :] = (W.T x.T), c-tiles of 128
                for ct in range(8):
                    pst = [psA.tile([128, 512], F32, tag="pa",
                                    name=f"psqk_{ct}_{st}")
                           for st in range(n_st)]
                    for dc in range(8):
                        wsl = wqkt[:, dc * 2 * CD + ct * 128:
                                   dc * 2 * CD + (ct + 1) * 128]
                        for st in range(n_st):
                            nc.tensor.matmul(
                                pst[st][:],
                                lhsT=wsl,
                                rhs=xt[:, dc * s + st * 512:
                                       dc * s + (st + 1) * 512],
                                start=(dc == 0), stop=False)
                    for st in range(n_st):
                        nc.tensor.matmul(
                            pst[st][:],
                            lhsT=bqk_t[0:1, ct * 128:(ct + 1) * 128],
                            rhs=ones_row[0:1, :],
                            start=False, stop=True)
                        nc.scalar.copy(
                            out=qkT[:, ct * s + st * 512:ct * s + (st + 1) * 512],
                            in_=pst[st][:])

                # v: natural [s, c] layout, s-chunks of 128, fused ones col
                for scg in range(n_sc // 4):
                    psv = [psA.tile([128, 512], F32, tag="pa",
                                    name=f"psv_{scg}_{i}")
                           for i in range(4)]
                    for dc in range(8):
                        for sci in range(4):
                            sc = scg * 4 + sci
                            nc.tensor.matmul(
                                psv[sci][:],
                                lhsT=xt[:, dc * s + sc * 128:
                                        dc * s + (sc + 1) * 128],
                                rhs=wvt[:, dc * CD:(dc + 1) * CD],
                                start=(dc == 0), stop=False)
                    for sci in range(4):
                        sc = scg * 4 + sci
                        nc.tensor.matmul(
                            psv[sci][:],
                            lhsT=ones_row[0:1, 0:128],
                            rhs=bv_t[0:1, :],
                            start=False, stop=True)
                        dst = vones[:, sc * VW:(sc + 1) * VW].rearrange(
                            "p (h e) -> p h e", e=DH + 1)[:, :, 0:DH]
                        src = psv[sci][:].rearrange("p (h e) -> p h e", e=DH)
                        nc.vector.tensor_copy(dst, src)

            # ---------------- phase B: attention ----------------
            with tc.tile_pool(name="poolB", bufs=1) as pb:
                ctxT = pb.tile([128, 4 * s], BF16, tag="ctxT")   # [512 c, s]
                with (
                    tc.tile_pool(name="poolE", bufs=6) as pe,
                    tc.tile_pool(name="poolBc", bufs=2) as pbc,
                    tc.tile_pool(name="psB_st", bufs=2, space="PSUM") as ps_st,
                    tc.tile_pool(name="psB_ctx", bufs=2, space="PSUM") as ps_ctx,
                ):
                    for hp in range(4):
                        h0, h1 = 2 * hp, 2 * hp + 1
                        kt_off = (4 + hp) * s   # K pair c-tile offset in qkT
                        qt_off = hp * s         # Q pair c-tile offset
                        rs_p = pbc.tile([2 * n_qh, fd_q], F32, tag="rsp",
                                        name=f"rs_{hp}")
                        rcp_p = pbc.tile([2 * n_qh, fd_q], F32, tag="rcpp",
                                         name=f"rcp_{hp}")
                        for qh in range(n_qh):
                            ctx = [ps_ctx.tile([DH + 1, fd_q], F32, tag="ctx",
                                               name=f"ctx_{hp}_{qh}_{i}")
                                   for i in range(2)]

                            def emit_pv(kc, est):
                                for hi, h in enumerate((h0, h1)):
                                    for n in range(n_qn):
                                        nc.tensor.matmul(
                                            ctx[hi][:, n * 512:(n + 1) * 512],
                                            lhsT=vones[:, kc * VW + h * (DH + 1):
                                                       kc * VW +
                                                       (h + 1) * (DH + 1)
                                                       ],
                                            rhs=est[hi][:, n * 512:(n + 1) * 512
                                                        ],
                                            start=(kc == 0),
                                            stop=(kc == n_kc - 1))

                            # software pipeline: PV for k-chunk kc-1 issues
                            # after the scores for kc, so the in-order PE
                            # queue never blocks on the exp+mask round trip
                            prev = None
                            for kc in range(n_kc):
                                est = []
                                for hi in range(2):
                                    pss = ps_st.tile([128, fd_q], F32, tag="st")
                                    r0, r1 = (0, 64) if hi == 0 else (64, 128)
                                    for n in range(n_qn):
                                        nc.tensor.matmul(
                                            pss[:, n * 512:(n + 1) * 512],
                                            lhsT=qkT[r0:r1,
                                                     kt_off + kc * 128:
                                                     kt_off + (kc + 1) * 128
                                                     ],
                                            rhs=qkT[r0:r1,
                                                    qt_off + qh * fd_q + n * 512:
                                                    qt_off + qh * fd_q +
                                                    (n + 1) * 512],
                                            start=True, stop=True,
                                            tile_position=(r0, 0))
                                    e = pe.tile([128, fd_q], BF16, tag="e")
                                    nc.scalar.activation(e[:], pss[:], EXP)
                                    msl = m01t[:, kc * s + qh * fd_q:
                                               kc * s + (qh + 1) * fd_q]
                                    nc.vector.tensor_tensor(
                                        e[:], e[:], msl, MULT)
                                    est.append(e)
                                if prev is not None:
                                    emit_pv(kc - 1, prev)
                                prev = est
                            emit_pv(n_kc - 1, prev)
                            # spill unnormalized ctx + rowsums
                            for hi, h in enumerate((h0, h1)):
                                stg = pbc.tile([1, fd_q], F32, tag="stg",
                                               name=f"rstg_{hp}_{qh}_{hi}")
                                nc.vector.tensor_copy(stg[:],
                                                      ctx[hi][DH:DH + 1, :])
                                nc.sync.dma_start(
                                    out=rs_p[hi * n_qh + qh:hi * n_qh + qh + 1, :],
                                    in_=stg[:])
                                nc.vector.tensor_copy(
                                    ctxT[hi * 64:(hi + 1) * 64,
                                         hp * s + qh * fd_q:
                                         hp * s + (qh + 1) * fd_q],
                                    ctx[hi][0:DH, :])

                        # normalize this pair: ctxT[c, q] *= 1/rowsum,
                        # broadcasting the bf16 reciprocal rows over the 64
                        # partitions of each head with a stride-0 DMA
                        with nc.allow_low_precision(
                                reason="recip feeds bf16 prob scale"):
                            nc.vector.reciprocal(rcp_p[:], rs_p[:])
                        rcpb = pbc.tile([2 * n_qh, fd_q], BF16, tag="rcpb",
                                        name=f"rcpb_{hp}")
                        nc.vector.tensor_copy(rcpb[:], rcp_p[:])
                        for qh in range(n_qh):
                            bcp = pbc.tile([128, fd_q], BF16, tag="bcp",
                                           name=f"bcp_{hp}_{qh}")
                            for hi in range(2):
                                r = rcpb[hi * n_qh + qh:hi * n_qh + qh + 1, :]
                                rep = bass.AP(r.tensor, r.offset,
                                              [list(r.ap[0]), [0, 64],
                                               [1, fd_q]])
                                nc.sync.dma_start(
                                    out=bcp[hi * 64:(hi + 1) * 64, :], in_=rep)
                            sl = ctxT[:, hp * s + qh * fd_q:
                                      hp * s + (qh + 1) * fd_q]
                            nc.vector.tensor_tensor(sl, sl, bcp[:], MULT)

                # ---------------- phase C: out projection ----------------
                with (
                    tc.tile_pool(name="poolC", bufs=2) as pc,
                    tc.tile_pool(name="poolCw", bufs=1) as pcw,
                    tc.tile_pool(name="psC", bufs=2, space="PSUM") as psC,
                ):
                    woutt = pcw.tile([128, 4 * D], BF16, tag="wout")
                    for ct in range(4):
                        nc.sync.dma_start(out=woutt[:, ct * D:(ct + 1) * D],
                                          in_=wout[ct * 128:(ct + 1) * 128, :])
                    for qc in range(n_sc):
                        ot = pc.tile([128, D], F32, tag="ot")
                        for n in range(2):
                            po = psC.tile([128, 512], F32, tag="po")
                            for ct in range(4):
                                nc.tensor.matmul(
                                    po[:],
                                    lhsT=ctxT[:, ct * s + qc * 128:
                                              ct * s + (qc + 1) * 128
                                              ],
                                    rhs=woutt[:, ct * D + n * 512:
                                              ct * D + (n + 1) * 512
                                              ],
                                    start=(ct == 0), stop=(ct == 3))
                            nc.scalar.copy(out=ot[:, n * 512:(n + 1) * 512],
                                           in_=po[:])
                        nc.sync.dma_start(
                            out=y[qc * 128:(qc + 1) * 128, :], in_=ot[:])

    _split_multiwait(nc)
    return nc


def _get_nc(s=S):
    if s not in _CACHE:
        _CACHE[s] = build_nc(s)
    return _CACHE[s]


def make_in_maps(x, W_qkv, b_qkv, W_out, mask, s=S):
    import ml_dtypes

    BF = ml_dtypes.bfloat16
    x = np.asarray(x, dtype=np.float32)
    W_qkv = np.asarray(W_qkv, dtype=np.float32)
    b_qkv = np.asarray(b_qkv, dtype=np.float32)
    W_out = np.asarray(W_out, dtype=np.float32)
    mask = np.asarray(mask)
    scale = 1.0 / np.sqrt(DH)
    m01 = np.ascontiguousarray((mask[0, 0] != 0).T.astype(BF))
    in_maps = []
    for c in range(NCORES):
        b, g = c // 2, c % 2
        wq = W_qkv[:, g * CD:(g + 1) * CD] * scale
        wk = W_qkv[:, D + g * CD:D + (g + 1) * CD]
        in_maps.append({
            "xT": np.ascontiguousarray(x[b].T.astype(BF)),
            "wqk": np.ascontiguousarray(
                np.concatenate([wq, wk], axis=1).astype(BF)),
            "wv": np.ascontiguousarray(
                W_qkv[:, 2 * D + g * CD:2 * D + (g + 1) * CD].astype(BF)),
            "bqk": np.ascontiguousarray(np.concatenate(
                [b_qkv[g * CD:(g + 1) * CD] * scale,
                 b_qkv[D + g * CD:D + (g + 1) * CD]])[None, :].astype(BF)),
            "bv": np.ascontiguousarray(
                b_qkv[2 * D + g * CD:2 * D + (g + 1) * CD][None, :].astype(BF)),
            "m01": m01,
            "wout": np.ascontiguousarray(
                W_out[g * CD:(g + 1) * CD, :].astype(BF)),
            "ones": np.ones((1, 512), dtype=BF),
        })
    return in_maps


def kernel(x, W_qkv, b_qkv, W_out, b_out, mask):
    from concourse.bass_utils import run_bass_kernel_spmd

    nc = _get_nc(S)
    in_maps = make_in_maps(x, W_qkv, b_qkv, W_out, mask, S)
    res = run_bass_kernel_spmd(nc, in_maps, list(range(NCORES)))
    b_out = np.asarray(b_out, dtype=np.float32)
    y = np.empty((B, S, D), dtype=np.float32)
    for b in range(B):
        y[b] = res.results[2 * b]["y"] + res.results[2 * b + 1]["y"] + b_out
    return y
